# revision 38
# baseline (speedup 1.0000x reference)
"""Trainium2 Bass kernel for MimiAttention (GQA + RoPE + causal softmax).

Problem: B=2, S=2048, H=1024, NH=16 q-heads, NKV=4 kv-heads, HD=64.
Sharding: 8 cores = 2 (batch) x 4 (kv-group).  Each core computes one batch's
attention for one GQA group (4 q-heads sharing 1 kv head) and the partial
o-projection for those heads; the host sums the 4 partials per batch.

v5 design (all bf16 matmuls, fp32 psum):
  * RoPE hat-trick (as baseline): wqk columns carry [q; q2] per head, qhat =
    proj * cs; khat = [k_rot; k_rot] via the J-fold matmul.
  * Scores transposed (scoresT[j, i]) per key-tile row, streamed through two
    ping-pong [128, 1024] PSUM feed regions; ONE exp per segment (<= 2 per
    row) minimizes the ACT fixed cost.  Causal diag masked in place on Pool.
  * Software pipeline: scores+exp for row r are issued BEFORE attnV of row
    r-1 so the PE never stalls behind the Pool mask / exp chain.
  * attnV accumulates [i, v|den] slices in 3 persistent psum banks
    (65-wide slices; col 64 = denominator via the ones column of v).
  * attn[i,c] -> aT[c,i] via PE transposes through the 1-bank work slot
    (pair 0 during head 2, pair 1 inline during head 3).
  * o-projection: 2 chunks per row during head 3 through the work bank,
    remainder after attention through 3 rotating psum slots; output DMAs
    batched 4 chunks each.
"""

import numpy as np
import ml_dtypes

B, S, H = 2, 2048, 1024
NH, NKV, HD = 16, 4, 64
G = NH // NKV            # 4 q-heads per kv head
THETA = 10000.0
N_CORES = 8

BF16 = ml_dtypes.bfloat16

NSB = S // 512           # 4 chunks of 512
NST = S // 128           # 16 tiles of 128
KC = H // 128            # 8 contraction chunks
SCALE = float(1.0 / np.sqrt(HD))


def _build_nc():
    import concourse.mybir as mybir
    import concourse.tile as tile
    from concourse.tile import add_dep_helper
    from concourse import bacc

    f32 = mybir.dt.float32
    bf16 = mybir.dt.bfloat16

    nc = bacc.Bacc("TRN2", target_bir_lowering=False)

    xTd = nc.dram_tensor("xT", [H, S], bf16, kind="ExternalInput")
    wqkd = nc.dram_tensor("wqkT", [H, 640], bf16, kind="ExternalInput")
    wk8d = nc.dram_tensor("wkT8", [128, KC, 128], bf16, kind="ExternalInput")
    wvd = nc.dram_tensor("wvT", [H, HD], bf16, kind="ExternalInput")
    csd = nc.dram_tensor("cs", [128, S], bf16, kind="ExternalInput")
    wod = nc.dram_tensor("woT", [G * HD, H], bf16, kind="ExternalInput")
    trid = nc.dram_tensor("trimask", [128, 128], bf16, kind="ExternalInput")
    djd = nc.dram_tensor("dupJ", [128, 128], bf16, kind="ExternalInput")
    idd = nc.dram_tensor("ident", [128, 128], bf16, kind="ExternalInput")
    oTd = nc.dram_tensor("oT", [H, S], bf16, kind="ExternalOutput")

    with tile.TileContext(nc) as tc:
        import contextlib
        ctx = contextlib.ExitStack()
        with ctx:
            consts = ctx.enter_context(tc.tile_pool(name="consts", bufs=1))
            acts = ctx.enter_context(tc.tile_pool(name="acts", bufs=1))
            ep = ctx.enter_context(tc.tile_pool(name="exps", bufs=3))
            rcp = ctx.enter_context(tc.tile_pool(name="rcp", bufs=6))
            otp = ctx.enter_context(tc.tile_pool(name="ot", bufs=3))
            pav = ctx.enter_context(
                tc.tile_pool(name="ps_av", bufs=1, space="PSUM"))
            pfa = ctx.enter_context(
                tc.tile_pool(name="ps_fa", bufs=1, space="PSUM"))
            pfb = ctx.enter_context(
                tc.tile_pool(name="ps_fb", bufs=1, space="PSUM"))
            pw = ctx.enter_context(
                tc.tile_pool(name="ps_w", bufs=1, space="PSUM"))

            # ---- input DMAs, ordered by first use: k weights + first xt
            # column block feed the k/q0 projections; the remaining xt lands
            # column-major so qhat chunks stream in order.
            xt_sb = consts.tile([128, KC, S], bf16, tag="xt")
            wqk_sb = consts.tile([128, KC, 640], bf16, tag="wqk")
            cs_sb = consts.tile([128, S], bf16, tag="cs")
            tri_sb = consts.tile([128, 128], bf16, tag="tri")
            dj_sb = consts.tile([128, 128], bf16, tag="dj")
            id_sb = consts.tile([128, 128], bf16, tag="id")
            wv_sb = consts.tile([128, KC, HD], bf16, tag="wv")
            wo_sb = consts.tile([128, 2, H], bf16, tag="wo")

            def xt_col(n):
                c = n * 512
                nc.sync.dma_start(
                    xt_sb[:, :, c:c + 512],
                    xTd[:, c:c + 512].rearrange("(kc p) m -> p kc m", p=128))

            def wqk_cols(c0, c1):
                nc.sync.dma_start(
                    wqk_sb[:, :, c0:c1],
                    wqkd[:, c0:c1].rearrange("(kc p) m -> p kc m", p=128))

            nc.sync.dma_start(wqk_sb[:, :, 512:640], wk8d[:, :, :])
            c = 0
            nc.sync.dma_start(
                xt_sb[:, 0:4, 0:512],
                xTd[0:512, 0:512].rearrange("(kc p) m -> p kc m", p=128))
            nc.sync.dma_start(
                xt_sb[:, 4:8, 0:512],
                xTd[512:1024, 0:512].rearrange("(kc p) m -> p kc m", p=128))
            nc.sync.dma_start(cs_sb, csd[:, :])
            nc.sync.dma_start(dj_sb, djd[:, :])
            wqk_cols(0, 128)            # q head 0
            xt_col(1)
            wqk_cols(128, 512)          # q heads 1-3
            nc.sync.dma_start(tri_sb, trid[:, :])
            xt_col(2)
            nc.sync.dma_start(wv_sb, wvd.rearrange("(kc p) m -> p kc m", p=128))
            xt_col(3)
            nc.sync.dma_start(wo_sb, wod.rearrange("(kc p) m -> p kc m", p=128))
            nc.sync.dma_start(id_sb, idd[:, :])

            qhat = [acts.tile([128, S], bf16, tag=f"qh{m}", name=f"qhat{m}")
                    for m in range(G)]
            khat = acts.tile([128, S], bf16, tag="khat")
            ktmp = acts.tile([128, S], bf16, tag="ktmp")
            v_sb = acts.tile([128, NST, HD + 1], bf16, tag="vsb")
            attn_n = acts.tile([128, NST, G * HD], bf16, tag="attn")
            aT = acts.tile([128, 2, S], bf16, tag="aT")

            avb = [pav.tile([128, w], f32, tag=f"av{b}", name=f"avb{b}")
                   for b, w in ((0, 455), (1, 455), (2, 130))]

            def av_slice(it):
                b, o = it // 7, (it % 7) * 65
                return avb[b][:, o:o + 65]

            seg_counter = [0]

            def feed_tile(idx, ln):
                # ping-pong exp-feed regions, allocated per segment so the
                # pool slot rotation provides the WAR chain
                if idx % 2 == 0:
                    return pfa.tile([128, ln], f32, tag="fA", name="feed",
                                    padded_shape=[128, 1024])
                return pfb.tile([128, ln], f32, tag="fB", name="feed",
                                padded_shape=[128, 1024])

            def proj_psum(m, n, ps):
                col = n * 512
                for kc in range(KC):
                    nc.tensor.matmul(
                        ps, wqk_sb[:, kc, m * 128:(m + 1) * 128],
                        xt_sb[:, kc, col:col + 512],
                        start=(kc == 0), stop=(kc == KC - 1))

            def q_chunk(h, n, ps=None):
                if ps is None:
                    ps = pw.tile([128, 512], f32, tag="w", name="psq")
                proj_psum(h, n, ps)
                col = n * 512
                nc.vector.tensor_mul(
                    qhat[h][:, col:col + 512], ps, cs_sb[:, col:col + 512])

            def k_proj(n, ps=None):
                if ps is None:
                    ps = pw.tile([128, 512], f32, tag="w", name="psk")
                proj_psum(G, n, ps)
                col = n * 512
                nc.vector.tensor_mul(
                    ktmp[:, col:col + 512], ps, cs_sb[:, col:col + 512])

            def k_fold(n, psf=None):
                col = n * 512
                if psf is None:
                    psf = pw.tile([128, 512], f32, tag="w", name="psf")
                nc.tensor.matmul(psf, dj_sb, ktmp[:, col:col + 512],
                                 start=True, stop=True)
                nc.vector.tensor_copy(khat[:, col:col + 512], psf)

            def k_chunk(n, ps=None, psf=None):
                k_proj(n, ps)
                k_fold(n, psf)

            def v_tiles(st0, nt):
                # project nt seq-tiles of v through one work-psum residency
                psv = pw.tile([128, nt, HD], f32, tag="w", name="psv",
                              padded_shape=[128, 4, HD])
                for t in range(nt):
                    st = st0 + t
                    for kc in range(KC):
                        nc.tensor.matmul(
                            psv[:, t, :],
                            xt_sb[:, kc, st * 128:(st + 1) * 128],
                            wv_sb[:, kc, :],
                            start=(t == 0 and kc == 0), stop=(kc == KC - 1),
                            skip_group_check=True)
                nc.vector.tensor_copy(
                    v_sb[:, st0:st0 + nt, 0:HD], psv)

            def transpose_group(hp, g4):
                # 4 slice transposes through one work-psum residency
                psx = pw.tile([128, 4, 128], bf16, tag="w", name="pst")
                for t in range(4):
                    it = g4 * 4 + t
                    nc.tensor.matmul(
                        psx[:, t, :], attn_n[:, it, hp * 128:(hp + 1) * 128],
                        id_sb, is_transpose=True,
                        start=(t == 0), stop=True, skip_group_check=True)
                nc.vector.tensor_copy(
                    aT[:, hp, g4 * 512:(g4 + 1) * 512], psx)

            # ---- prologue
            nc.gpsimd.memset(v_sb[:, :, HD:HD + 1], 1.0)
            k_proj(0, ps=feed_tile(0, 512))
            q_chunk(0, 0, ps=feed_tile(1, 512))
            k_fold(0, psf=pw.tile([128, 512], f32, tag="w", name="psf0"))
            q_chunk(0, 1, ps=pw.tile([128, 512], f32, tag="w", name="psq0"))
            v_tiles(0, 2)
            seg_counter[0] = 2

            def scores_row(h, jt, et, segs=None, cbs=None):
                lo = jt * 128
                cols = S - lo
                lhsT = khat[:, lo:lo + 128]
                if segs is None:
                    segs = [(lo, min(1024, cols))]
                    if cols > 1024:
                        segs.append((lo + 1024, cols - 1024))
                for si, (off, ln) in enumerate(segs):
                    region = feed_tile(seg_counter[0], ln)
                    seg_counter[0] += 1
                    done = 0
                    while done < ln:
                        cl = min(512, ln - done)
                        nc.tensor.matmul(
                            region[:, done:done + cl], lhsT,
                            qhat[h][:, off + done:off + done + cl],
                            start=True, stop=True)
                        done += cl
                    nc.scalar.activation(
                        et[:, off:off + ln], region[:, 0:ln],
                        mybir.ActivationFunctionType.Exp, scale=SCALE)
                    if cbs is not None and si in cbs:
                        cbs[si]()
                # causal mask on diag tile: Pool, hidden by the pipeline
                nc.gpsimd.tensor_mul(et[:, lo:lo + 128],
                                     et[:, lo:lo + 128], tri_sb)

            attnv_state = {}   # h -> bank_first dict

            def attnv_row(h, jt, et):
                bank_first = attnv_state.setdefault(h, {})
                b1_hi = min(jt + 7, NST - 1)
                order = list(range(b1_hi, jt - 1, -1)) + \
                    list(range(NST - 1, b1_hi, -1))
                for it in order:
                    b = it // 7
                    first = jt == 0 and b not in bank_first
                    mm = nc.tensor.matmul(
                        av_slice(it), et[:, it * 128:(it + 1) * 128],
                        v_sb[:, jt, :],
                        start=first, stop=(it == jt),
                        skip_group_check=True)
                    if first:
                        bank_first[b] = mm
                    elif jt == 0:
                        add_dep_helper(mm.ins, bank_first[b].ins,
                                       sync=False,
                                       reason="bank clear first")
                pso = av_slice(jt)
                rc = rcp.tile([128, 1], f32, tag="rc", name="rc")
                nc.vector.reciprocal(rc, pso[:, HD:HD + 1])
                nc.vector.tensor_scalar_mul(
                    attn_n[:, jt, h * HD:(h + 1) * HD], pso[:, 0:HD], rc)

            # oproj -------------------------------------------------------
            oproj_pending = [(g, hc) for g in range(NSB) for hc in range(KC)]
            ot_state = {}

            def oproj_chunk(ps, drain_eng):
                g, hc = oproj_pending.pop(0)
                col = g * 512
                for kc2 in range(2):
                    nc.tensor.matmul(
                        ps, wo_sb[:, kc2, hc * 128:(hc + 1) * 128],
                        aT[:, kc2, col:col + 512],
                        start=(kc2 == 0), stop=(kc2 == 1))
                if hc % 4 == 0:
                    ot_state[g] = otp.tile([128, 4, 512], bf16, tag="otb",
                                           name="otb")
                ot = ot_state[g]
                if drain_eng == 0:
                    nc.vector.tensor_copy(ot[:, hc % 4, :], ps)
                elif drain_eng == 1:
                    nc.scalar.copy(ot[:, hc % 4, :], ps)
                else:
                    nc.vector.tensor_copy(ot[:, hc % 4, 0:256], ps[:, 0:256])
                    nc.scalar.copy(ot[:, hc % 4, 256:512], ps[:, 256:512])
                if hc % 4 == 3:
                    r0 = (hc // 4) * 512
                    nc.sync.dma_start(
                        oTd[r0:r0 + 512, col:col + 512].rearrange(
                            "(c p) m -> p c m", p=128), ot)

            # ---- main pipelined loop ------------------------------------
            seq = [(h, jt) for h in range(G) for jt in range(NST)]
            prev = None
            for (h, jt) in seq:
                et = ep.tile([128, S], bf16, tag="e", name=f"e{h}_{jt}")
                if h == 0 and jt == 0:
                    def _row0cb():
                        ps2 = feed_tile(seg_counter[0], 1024)
                        seg_counter[0] += 1
                        proj_psum(0, 2, ps2[:, 0:512])
                        proj_psum(0, 3, ps2[:, 512:1024])
                        nc.vector.tensor_mul(
                            qhat[0][:, 1024:2048], ps2, cs_sb[:, 1024:2048])
                    scores_row(h, jt, et, cbs={0: _row0cb})
                else:
                    scores_row(h, jt, et)
                if prev is not None:
                    attnv_row(*prev)
                prev = (h, jt, et)

                # interleaved producer work, at most ~one work-slot per row
                if h == 0:
                    if jt == 1:
                        v_tiles(2, 2)
                    if jt in (4, 8, 12):
                        v_tiles(jt + 0, 4)
                    if jt in (2, 6, 10):
                        k_proj(jt // 4 + 1)
                    if jt in (3, 7, 11):
                        k_fold((jt + 1) // 4)
                    if jt in (5, 9, 13, 14):
                        q_chunk(1, (5, 9, 13, 14).index(jt))
                if h in (1, 2) and jt in (1, 4, 7, 10):
                    q_chunk(h + 1, (jt - 1) // 3)
                if h == 2 and jt in (3, 7, 11, 15):
                    transpose_group(0, jt // 4)
                if h == 3:
                    if jt in (5, 9, 13):
                        transpose_group(1, (jt - 5) // 4)
                    if jt >= 5 and oproj_pending and \
                            oproj_pending[0][0] * 4 + 5 <= jt:
                        ps = pw.tile([128, 512], f32, tag="w", name="psow")
                        oproj_chunk(ps, drain_eng=0)
                    if jt >= 8 and oproj_pending and \
                            oproj_pending[0][0] * 4 + 5 <= jt:
                        ps = pav.tile([128, 512], f32, tag="av0",
                                      name="psoa")
                        oproj_chunk(ps, drain_eng=1 if jt >= 12 else 0)
                    if jt == 15 and oproj_pending and \
                            oproj_pending[0][0] * 4 + 5 <= jt:
                        ps = pav.tile([128, 512], f32, tag="av1",
                                      name="psob")
                        oproj_chunk(ps, drain_eng=1)

            # flush: last attnV row + final transposes + remaining oproj
            attnv_row(*prev)
            transpose_group(1, 3)
            ti = 0
            slots = ["w", "fA", "fB", "av0", "av1"]
            pools = {"w": pw, "fA": pfa, "fB": pfb, "av0": pav, "av1": pav}
            while oproj_pending:
                tag = slots[ti % len(slots)]
                ps = pools[tag].tile([128, 512], f32, tag=tag, name="psot")
                oproj_chunk(ps, drain_eng=ti % 2)
                ti += 1

    nc.finalize()
    return nc


def _host_inputs(hidden_states, position_ids, wq, wk, wv, wo):
    """Build the 8 per-core input maps."""
    def w2_of(w):
        # w: [64, H] rows of one head; returns sign-permuted rows
        w2 = np.empty_like(w)
        w2[:32] = -w[32:64]
        w2[32:] = w[:32]
        return w2

    trimask = np.triu(np.ones((128, 128), np.float32)).astype(BF16)
    dupJ = np.zeros((128, 128), np.float32)
    for p in range(128):
        dupJ[p, p % 64] = 1.0
        dupJ[p, p % 64 + 64] = 1.0
    dupJ = dupJ.astype(BF16)
    ident = np.eye(128, dtype=np.float32).astype(BF16)

    in_maps = []
    for core in range(N_CORES):
        b, kv = core // NKV, core % NKV
        xT = np.ascontiguousarray(hidden_states[b].T).astype(BF16)

        cols = []
        for i in range(G):
            h = kv * G + i
            wqh = wq[h * HD:(h + 1) * HD]
            cols.append(wqh.T)
            cols.append(w2_of(wqh).T)
        wkh = wk[kv * HD:(kv + 1) * HD]
        cols.append(wkh.T)
        cols.append(w2_of(wkh).T)
        wqkT = np.ascontiguousarray(np.concatenate(cols, axis=1)).astype(BF16)

        wvT = np.ascontiguousarray(wv[kv * HD:(kv + 1) * HD].T).astype(BF16)
        wkT8 = np.ascontiguousarray(
            wqkT[:, 512:640].reshape(KC, 128, 128).transpose(1, 0, 2)
        ).astype(BF16)
        woT = np.ascontiguousarray(
            wo[:, kv * G * HD:(kv + 1) * G * HD].T).astype(BF16)

        inv = 1.0 / (THETA ** (np.arange(0, HD, 2, dtype=np.float32) / HD))
        freqs = position_ids[b].astype(np.float32)[:, None] * inv[None, :]
        emb = np.concatenate([freqs, freqs], axis=-1)       # [S, 64]
        cs = np.concatenate([np.cos(emb).T, np.sin(emb).T], axis=0)  # [128, S]
        cs = np.ascontiguousarray(cs).astype(BF16)

        in_maps.append({
            "xT": xT, "wqkT": wqkT, "wvT": wvT, "cs": cs, "woT": woT,
            "trimask": trimask, "dupJ": dupJ, "ident": ident,
            "wkT8": wkT8,
        })
    return in_maps


_NC_CACHE = {}


def run_cores(in_maps, trace=False, trace_kwargs=None):
    from concourse.bass_utils import run_bass_kernel_spmd
    if "nc" not in _NC_CACHE:
        _NC_CACHE["nc"] = _build_nc()
    nc = _NC_CACHE["nc"]
    return run_bass_kernel_spmd(
        nc, in_maps, core_ids=list(range(N_CORES)),
        trace=trace, **(trace_kwargs or {}))


def kernel(hidden_states, attention_mask, position_ids, wq, wk, wv, wo):
    hidden_states = np.asarray(hidden_states, dtype=np.float32)
    position_ids = np.asarray(position_ids)
    wq = np.asarray(wq, dtype=np.float32)
    wk = np.asarray(wk, dtype=np.float32)
    wv = np.asarray(wv, dtype=np.float32)
    wo = np.asarray(wo, dtype=np.float32)

    in_maps = _host_inputs(hidden_states, position_ids, wq, wk, wv, wo)
    res = run_cores(in_maps)

    out = np.zeros((B, S, H), np.float32)
    for core in range(N_CORES):
        b = core // NKV
        out[b] += res.results[core]["oT"].T.astype(np.float32)
    return out
